# revision 27
# baseline (speedup 1.0000x reference)
"""Trainium2 Bass kernel for per-pixel dot-product attention.

Reference op (per pixel, over C=80 channels split q/k/v = 8/64/8):
    qk[v] = sum_k q[k] * K[k, v] / sqrt(8)
    attn  = softmax(qk over v)
    out[v] = attn[v] * V[v]

Strategy: pure data-parallel over 8 NeuronCores — core i handles batch
i//2, H-rows half (i%2).  ~46.1 MB HBM traffic/core at the ~358 GB/s
per-core HBM limit puts the DMA floor at ~129 us; the kernel is DMA-
bound, so everything is organized around keeping the 16 SDMA engines
streaming:

  * super-chunk pixel layout [512, 384, 128] columns/partition so load
    descriptors are 2KB/1.5KB (≈ HBM line rate; the v1 kernel's 1KB
    descriptors reached only ~91%), with the small last chunk bounding
    the end-of-kernel compute tail;
  * K streamed on the sync HWDGE ring in 8-channel pieces; ScalarE
    downcasts to bf16 and DVE runs the multiply-accumulate at the
    bf16 2x (single-port) rate;
  * software-pipelined emission: chunk s-1's softmax/output phase is
    emitted inside chunk s's K-loop so its ACT exps never head-of-line
    block chunk s's staging conversions, and its stores are issued
    after chunk s's load emission;
  * quarter-subtiled bf16 output phase (exp -> pairwise sums ->
    reciprocal_approx_fast -> *V -> *1/s) with SWDGE dtype-cast stores
    (bf16 SBUF -> f32 HBM), all r handling kept on DVE so the only
    cross-engine handoff per subtile is the exp;
  * the last chunk's output phase runs in f32 with HWDGE stores and a
    column-subtiled final multiply-accumulate to minimize the
    post-last-load critical chain.

Measured ~150 us/NEFF warm median (runs vary ~150-167 us with shared-
device noise) vs 161 us for the v1 kernel; output rel-l2 vs the f32
reference ≈ 5.5e-3 (bf16 qk tree + bf16 softmax/output path).
"""

import numpy as np

NK = 8
NV = 8
C = NK + NK * NV + NV  # 80
B, H, W = 4, 512, 512
N_CORES = 8
ROWS = H // 2            # rows per core
PIX = ROWS * W           # pixels per core (131072)
NCHUNK = 8               # chunks per core
_SCALE = 1.0 / float(np.sqrt(NK))


def _ensure_path():
    import sys
    p = "/opt/trn_rl_repo"
    if p not in sys.path:
        sys.path.insert(0, p)


def build_nc(pix=PIX, nchunk=NCHUNK, recip_on_act=False, bf16_tree=False,
             k_splits=2, inplace_tree=False, lean_bufs=False, conv_k=False,
             chunk_cols=None, split_rings=False, stage_k=False, direct_b=False,
             n_conv_blocks=4):
    """Build the per-core Bass program for a (80, pix) f32 shard.

    All tensor_tensor work runs on DVE (GPSIMD shares an SBUF port with DVE
    and the two engines serialize, so Pool offload is a net loss).  K streams
    in on the sync HWDGE ring in `k_splits` pieces (compute starts after the
    first piece); q/v loads and the output go on the scalar ring.  With
    `inplace_tree` the l1/l2 add-tree levels write back into the prod tile
    (strictly trailing writes, single-engine serial) to fit ncol=256 in SBUF.
    """
    _ensure_path()
    import concourse.tile as tile
    from concourse import bacc, mybir

    f32 = mybir.dt.float32
    mid = mybir.dt.bfloat16 if bf16_tree else f32
    if chunk_cols is None:
        npix = pix // nchunk
        assert npix % 128 == 0
        chunk_cols = [npix // 128] * nchunk
    assert sum(chunk_cols) * 128 == pix

    nc = bacc.Bacc("TRN2", target_bir_lowering=False, debug=False)
    x = nc.dram_tensor("x", [C, pix], f32, kind="ExternalInput")
    y = nc.dram_tensor("y", [NV, pix], f32, kind="ExternalOutput")

    qv_bufs = 1 if lean_bufs else 2
    pipe_bufs = 1 if lean_bufs else 2
    # deferred output DMAs: emit chunk j's store after chunk j+1's input
    # triggers so it never head-of-line-blocks loads on its ring
    pending_out = []

    def flush_out():
        for args in pending_out:
            nc.scalar.dma_start(**args)
        pending_out.clear()

    with tile.TileContext(nc) as tc:
        with (
            tc.tile_pool(name="inp", bufs=2) as in_pool,
            tc.tile_pool(name="work", bufs=1) as work_pool,
            tc.tile_pool(name="pipe", bufs=pipe_bufs) as pipe_pool,
        ):
            off = 0
            for j, ncol in enumerate(chunk_cols):
                npix = 128 * ncol
                q_t = in_pool.tile([128, NK * ncol], f32, name=f"q{j}", tag="q", bufs=qv_bufs)
                v_t = in_pool.tile([128, NV * ncol], f32, name=f"v{j}", tag="v", bufs=qv_bufs)

                # K channel layout is k-major (channel NK + k*NV + v), matching
                # the prod block order; split loads so prod can start early
                sp_ch = NK * NV // k_splits
                if conv_k:  # match the B-half-first compute order
                    dma_order = [h for h in range(k_splits) if h * sp_ch >= NK * NV // 2] + \
                                [h for h in range(k_splits) if h * sp_ch < NK * NV // 2]
                else:
                    dma_order = list(range(k_splits))
                k_stages = {}
                if not stage_k:
                    k_t = in_pool.tile([128, NK * NV * ncol], f32, name=f"k{j}", tag="k")
                    k4 = k_t.rearrange("p (k v x) -> p k v x", k=NK, v=NV)

                def emit_k(h):
                    if stage_k:
                        kst = in_pool.tile([128, sp_ch * ncol], f32,
                                           name=f"kst{j}_{h}", tag="kst", bufs=4)
                        k_stages[h] = kst
                        dst = kst.rearrange("p (c x) -> p c x", c=sp_ch)
                    else:
                        dst = k_t.rearrange("p (c x) -> p c x", c=NK * NV)[
                            :, h * sp_ch:(h + 1) * sp_ch]
                    on_sync = (h * sp_ch >= NK * NV // 2) or not split_rings
                    ring = nc.sync if on_sync else nc.scalar
                    ring.dma_start(
                        out=dst,
                        in_=x[NK + h * sp_ch:NK + (h + 1) * sp_ch, off:off + npix]
                        .rearrange("c (p x) -> p c x", p=128),
                    )

                # ramp: the first K piece goes before q so its descriptors
                # generate first; q loads in halves, upper (k=4..7, feeding
                # the direct B products) first
                emit_k(dma_order[0])
                q_ring = nc.scalar if split_rings else nc.sync
                for cl, ch_ in ((NK // 2, NK), (0, NK // 2)):
                    q_ring.dma_start(
                        out=q_t.rearrange("p (c x) -> p c x", c=NK)[:, cl:ch_],
                        in_=x[cl:ch_, off:off + npix].rearrange("c (p x) -> p c x", p=128),
                    )
                for h in dma_order[1:]:
                    emit_k(h)
                nc.sync.dma_start(
                    out=v_t.rearrange("p (c x) -> p c x", c=NV),
                    in_=x[NK + NK * NV:C, off:off + npix]
                    .rearrange("c (p x) -> p c x", p=128),
                )
                # previous chunk's output, behind this chunk's input triggers
                flush_out()

                # prod[k,v] = q[k] * K[k,v]   (one broadcast multiply per K piece)
                sp_k = NK // k_splits
                if conv_k:
                    # ScalarE downcasts K into the prod tiles and Q into a small
                    # bf16 tile; DVE then multiplies in-place at bf16 2x rate.
                    # prod is split into two half-tiles: B frees after l1, so
                    # the next chunk's conversions overlap this chunk's tail.
                    assert bf16_tree and inplace_tree
                    half = NK // 2
                    prodA = work_pool.tile([128, half * NV * ncol], mid,
                                           name=f"prodA{j}", tag="prodA")
                    prodB = work_pool.tile([128, half * NV * ncol], mid,
                                           name=f"prodB{j}", tag="prodB")
                    p4A = prodA.rearrange("p (k v x) -> p k v x", k=half, v=NV)
                    p4B = prodB.rearrange("p (k v x) -> p k v x", k=half, v=NV)
                    q_bf = work_pool.tile([128, NK * ncol], mybir.dt.bfloat16,
                                          name=f"qbf{j}", tag="qbf")
                    nc.scalar.activation(q_bf, q_t, mybir.ActivationFunctionType.Copy)
                    q_b = (
                        q_bf.rearrange("p (k x) -> p k x", k=NK)
                        .unsqueeze(2)
                        .broadcast_to((128, NK, NV, ncol))
                    )

                    def pslice(kl, kh):  # view of prod blocks [kl, kh)
                        if kh <= half:
                            return p4A[:, kl:kh]
                        assert kl >= half
                        return p4B[:, kl - half:kh - half]

                    # emit B-half first (its tile frees earliest, after l1),
                    # then direct A-blocks, then converted A-blocks — the
                    # direct ones give DVE an ACT-free runway at each boundary
                    ncb = n_conv_blocks
                    a_blocks = [h for h in range(k_splits) if h * sp_k < half]
                    order = [h for h in range(k_splits) if h * sp_k >= half] + \
                            sorted(a_blocks, key=lambda h: h * sp_k < ncb)
                    if direct_b:
                        q_bf32 = (
                            q_t.rearrange("p (k x) -> p k x", k=NK)
                            .unsqueeze(2)
                            .broadcast_to((128, NK, NV, ncol))
                        )
                    for h in order:
                        kl, kh = h * sp_k, (h + 1) * sp_k
                        pv = pslice(kl, kh)
                        if stage_k:
                            src = k_stages[h].rearrange("p (k v x) -> p k v x",
                                                        k=sp_k, v=NV)
                        else:
                            src = k4[:, kl:kh]
                        if direct_b and kl >= ncb:
                            # B half: direct f32 multiply (bf16 out) — no ACT
                            # dependency, so DVE starts as soon as K lands;
                            # ACT meanwhile pre-converts the A half
                            nc.vector.tensor_tensor(
                                pv, q_bf32[:, kl:kh], src, mybir.AluOpType.mult
                            )
                        else:
                            nc.scalar.activation(pv, src,
                                                 mybir.ActivationFunctionType.Copy)
                            nc.vector.tensor_tensor(
                                pv, q_b[:, kl:kh], pv, mybir.AluOpType.mult
                            )
                    # tree: l1 = A + B -> A; l2, qk within A
                    nc.vector.tensor_tensor(p4A, p4A, p4B, mybir.AluOpType.add)
                    l24 = p4A[:, 0:2]
                    nc.vector.tensor_tensor(l24, p4A[:, 0:2], p4A[:, 2:4], mybir.AluOpType.add)
                else:
                    prod = work_pool.tile([128, NK * NV * ncol], mid, name=f"prod{j}", tag="prod")
                    p4 = prod.rearrange("p (k v x) -> p k v x", k=NK, v=NV)
                    q_b = (
                        q_t.rearrange("p (k x) -> p k x", k=NK)
                        .unsqueeze(2)
                        .broadcast_to((128, NK, NV, ncol))
                    )
                    for h in range(k_splits):
                        kl, kh = h * sp_k, (h + 1) * sp_k
                        nc.vector.tensor_tensor(
                            p4[:, kl:kh], q_b[:, kl:kh], k4[:, kl:kh], mybir.AluOpType.mult
                        )

                    # sum over k (outer block index): 3-level pairwise tree (all DVE)
                    if inplace_tree:
                        # l1 -> prod[k 0:4], l2 -> prod[k 0:2]: strictly in-place
                        # (out == in0), serial on DVE
                        l14 = p4[:, 0:4]
                        nc.vector.tensor_tensor(l14, p4[:, 0:4], p4[:, 4:8], mybir.AluOpType.add)
                        l24 = p4[:, 0:2]
                        nc.vector.tensor_tensor(l24, l14[:, 0:2], l14[:, 2:4], mybir.AluOpType.add)
                    else:
                        l1 = work_pool.tile([128, 4 * NV * ncol], mid, name=f"l1_{j}", tag="l1")
                        l14 = l1.rearrange("p (k v x) -> p k v x", k=4, v=NV)
                        nc.vector.tensor_tensor(l14, p4[:, 0:4], p4[:, 4:8], mybir.AluOpType.add)
                        l2 = work_pool.tile([128, 2 * NV * ncol], mid, name=f"l2_{j}", tag="l2")
                        l24 = l2.rearrange("p (k v x) -> p k v x", k=2, v=NV)
                        nc.vector.tensor_tensor(l24, l14[:, 0:2], l14[:, 2:4], mybir.AluOpType.add)
                # qk shares t1's slot: qk dies at exp, t1 is born after exp
                qk = pipe_pool.tile([128, NV * ncol], mid, name=f"qk{j}", tag="t1", bufs=1)
                qk4 = qk.rearrange("p (v x) -> p v x", v=NV).unsqueeze(1)
                nc.vector.tensor_tensor(qk4, l24[:, 0:1], l24[:, 1:2], mybir.AluOpType.add)

                # e = exp(qk / sqrt(NK)); softmax denominators over v
                e_bufs = 1 if max(chunk_cols) > 256 else 2
                e = pipe_pool.tile([128, NV * ncol], f32, name=f"e{j}", tag="e", bufs=e_bufs)
                nc.scalar.activation(e, qk, mybir.ActivationFunctionType.Exp, scale=_SCALE)
                t1 = pipe_pool.tile([128, 4 * ncol], f32, name=f"t1_{j}", tag="t1", bufs=1)
                nc.vector.tensor_tensor(t1, e[:, 0:4 * ncol], e[:, 4 * ncol:], mybir.AluOpType.add)
                # t2 / s / r share one scratch tile (padding control)
                sc = pipe_pool.tile([128, 4 * ncol], f32, name=f"sc{j}", tag="sc", bufs=1)
                t2 = sc[:, 0:2 * ncol]
                nc.vector.tensor_tensor(t2, t1[:, 0:2 * ncol], t1[:, 2 * ncol:], mybir.AluOpType.add)
                s = sc[:, 2 * ncol:3 * ncol]
                nc.vector.tensor_tensor(s, t2[:, 0:ncol], t2[:, ncol:], mybir.AluOpType.add)
                r = sc[:, 3 * ncol:4 * ncol]
                if recip_on_act:
                    # r = exp(-ln s): needs two ACT table sets (thrash) but
                    # stays off the DVE critical path
                    ls = sc[:, 0:ncol]
                    nc.scalar.activation(ls, s, mybir.ActivationFunctionType.Ln)
                    nc.scalar.activation(r, ls, mybir.ActivationFunctionType.Exp, scale=-1.0)
                else:
                    nc.vector.reciprocal(r, s)

                # out[v] = e[v] * V[v] * r  (both multiplies in-place on e; DVE
                # executes them after the t-tree reads of e).  The stride-0
                # broadcast operand goes in in0 — a stride-0 in1 runs at half
                # rate on DVE.
                e3 = e.rearrange("p (v x) -> p v x", v=NV)
                v3 = v_t.rearrange("p (v x) -> p v x", v=NV)
                r_b = r.unsqueeze(1).broadcast_to((128, NV, ncol))
                nc.vector.tensor_tensor(e3, e3, v3, mybir.AluOpType.mult)
                nc.vector.tensor_tensor(e3, r_b, e3, mybir.AluOpType.mult)
                # one output DMA per chunk on the scalar HWDGE ring (deferred)
                pending_out.append(dict(
                    out=y[0:NV, off:off + npix].rearrange("c (p x) -> p c x", p=128),
                    in_=e.rearrange("p (c x) -> p c x", c=NV),
                ))
                off += npix
            flush_out()
    nc.compile()
    return nc


def build_nc_v2(chunk_cols=(512, 512), out_halves=2, kst_bufs=3, e_bufs=1,
                acc_bufs=2, e_bf16=False, defer_stores=False, swdge_k=False,
                kbf_bufs=2, swdge_qv=False, qv_bufs=2, fast_r=True,
                dve_rb=True):
    """v2: super-chunks with 2KB DMA descriptors + streamed-K accumulation.

    Layout per core: (80, 131072) f32 shard, pixels partition-major within
    each super-chunk (partition p owns `ncol` consecutive pixels), so every
    HBM descriptor is ncol*4 bytes (2KB at ncol=512 — full HBM line rate vs
    ~91% at the v1 1KB descriptors).  K streams on the sync ring in 8-channel
    pieces; ScalarE downcasts each piece (and q) to bf16, DVE runs the
    multiply-accumulate at bf16 2x rate.  The softmax/output phase is split
    into `out_halves` column subtiles so the final store pipeline keeps DMA
    busy through the tail.
    """
    _ensure_path()
    import concourse.tile as tile
    from concourse import bacc, mybir

    f32 = mybir.dt.float32
    bf16 = mybir.dt.bfloat16
    edt = bf16 if e_bf16 else f32
    chunk_cols = list(chunk_cols)
    assert sum(chunk_cols) * 128 == PIX

    nc = bacc.Bacc("TRN2", target_bir_lowering=False, debug=False)
    x = nc.dram_tensor("x", [C, PIX], f32, kind="ExternalInput")
    y = nc.dram_tensor("y", [NV, PIX], f32, kind="ExternalOutput")

    with tile.TileContext(nc) as tc:
        with (
            tc.tile_pool(name="inp", bufs=2) as in_pool,
            tc.tile_pool(name="work", bufs=1) as work_pool,
        ):
            offs = []
            o = 0
            for ncol in chunk_cols:
                offs.append(o)
                o += 128 * ncol

            pending_stores = []

            def flush_stores():
                for args in pending_stores:
                    if e_bf16:
                        nc.gpsimd.dma_start(**args)  # SWDGE casts bf16 -> f32
                    else:
                        nc.scalar.dma_start(**args)
                pending_stores.clear()

            def emit_output_phase(s, acc, v_t, ncol, last):
                """softmax + output for chunk s, subtiled over columns.

                Big chunks run the e-path in bf16 (DVE 2x) with SWDGE-cast
                stores.  The last (small, latency-critical) chunk runs it in
                f32: at tiny FD the op costs are similar and f32 drops the
                rbf ACT hop and the SWDGE dispatch/completion latency from
                the end-of-kernel critical chain (HWDGE store, no cast).
                """
                cdt = f32 if last else edt
                off, npix = offs[s], 128 * ncol
                acc_v = acc.rearrange("p (v x) -> p v x", v=NV)
                e = work_pool.tile([128, NV * ncol], cdt, name=f"e{s}",
                                   tag="e" if cdt is edt else "ef", bufs=e_bufs)
                e_v = e.rearrange("p (v x) -> p v x", v=NV)
                v3 = v_t.rearrange("p (c x) -> p c x", c=NV)
                hw = ncol // out_halves
                for h in range(out_halves):
                    lo, hi = h * hw, (h + 1) * hw
                    e_h = e_v[:, :, lo:hi]
                    nc.scalar.activation(e_h, acc_v[:, :, lo:hi],
                                         mybir.ActivationFunctionType.Exp,
                                         scale=_SCALE)
                    t1 = work_pool.tile([128, 4 * hw], cdt, name=f"t1_{s}{h}",
                                        tag="t1", bufs=1)
                    t13 = t1.rearrange("p (c x) -> p c x", c=4)
                    nc.vector.tensor_tensor(t13, e_v[:, 0:4, lo:hi],
                                            e_v[:, 4:8, lo:hi], mybir.AluOpType.add)
                    t2t = work_pool.tile([128, 2 * hw], cdt, name=f"t2_{s}{h}",
                                         tag="t2", bufs=1)
                    t2 = t2t.rearrange("p (c x) -> p c x", c=2)
                    nc.vector.tensor_tensor(t2, t13[:, 0:2], t13[:, 2:4],
                                            mybir.AluOpType.add)
                    sc4 = work_pool.tile([128, 3 * hw], f32, name=f"sc{s}{h}",
                                         tag="sc", bufs=1)
                    ssum = sc4[:, 0:hw]
                    nc.vector.tensor_tensor(ssum, t2[:, 0], t2[:, 1],
                                            mybir.AluOpType.add)
                    r = sc4[:, hw:2 * hw]
                    if fast_r:
                        # s = sum of 8 exp() values, strictly positive and in
                        # range, so the fast approx (18 bits ≫ bf16) is safe
                        nc.vector.reciprocal_approx_fast(r, ssum)
                    else:
                        nc.vector.reciprocal(r, ssum)
                    if cdt is not f32:
                        rb_t = work_pool.tile([128, hw], edt, name=f"rb{s}{h}",
                                              tag="rb", bufs=1)
                        if dve_rb:
                            # DVE-side cast: keeps the r hop off ACT so the
                            # only cross-engine handoff per subtile is the exp
                            nc.vector.tensor_scalar_mul(rb_t, r, 1.0)
                        else:
                            nc.scalar.activation(
                                rb_t, r, mybir.ActivationFunctionType.Copy)
                        r_b = rb_t.unsqueeze(1).broadcast_to((128, NV, hw))
                    else:
                        r_b = r.unsqueeze(1).broadcast_to((128, NV, hw))
                    nc.vector.tensor_tensor(e_h, e_h, v3[:, :, lo:hi],
                                            mybir.AluOpType.mult)
                    nc.vector.tensor_tensor(e_h, r_b, e_h, mybir.AluOpType.mult)
                    st = dict(
                        out=y[0:NV, off:off + npix]
                        .rearrange("c (p x) -> p c x", p=128)[:, :, lo:hi],
                        in_=e_h,
                    )
                    if last:
                        nc.scalar.dma_start(**st)
                    else:
                        # issued after the NEXT chunk's load emission so the
                        # store's wait-on-DVE never head-of-line-blocks the
                        # load descriptor stream on its engine
                        pending_stores.append(st)

            # software-pipelined emission: chunk s-1's softmax/output phase is
            # emitted inside chunk s's K-loop (after the second conversion) so
            # its ACT exps / store dispatches slot into the conv idle gaps
            # instead of head-of-line-blocking chunk s's staging conversions.
            prev = None  # (s, acc, ev_t, ncol) awaiting output phase
            for s, ncol in enumerate(chunk_cols):
                off, npix = offs[s], 128 * ncol
                x_sl = lambda c0, c1: (
                    x[c0:c1, off:off + npix].rearrange("c (p x) -> p c x", p=128))
                # ---- q
                q_bf = work_pool.tile([128, NK * ncol], bf16, name=f"qbf{s}",
                                      tag="qbf", bufs=2)
                if swdge_k or swdge_qv:
                    nc.gpsimd.dma_start(
                        out=q_bf.rearrange("p (c x) -> p c x", c=NK),
                        in_=x_sl(0, NK))
                else:
                    q_t = in_pool.tile([128, NK * ncol], f32, name=f"q{s}",
                                       tag="q", bufs=qv_bufs)
                    nc.sync.dma_start(
                        out=q_t.rearrange("p (c x) -> p c x", c=NK), in_=x_sl(0, NK))
                    nc.scalar.activation(q_bf, q_t,
                                         mybir.ActivationFunctionType.Copy)
                qbf3 = q_bf.rearrange("p (k x) -> p k x", k=NK)

                def emit_v_load():
                    # v is consumed only at e*v in the NEXT chunk's window, so
                    # it loads after this chunk's K pieces — for the last chunk
                    # that lets the matvec+softmax chain run during v's
                    # transfer instead of after it.
                    if e_bf16 and s + 1 < len(chunk_cols):
                        ev = work_pool.tile([128, NV * ncol], bf16,
                                            name=f"vbf{s}", tag="vbf", bufs=2)
                        if swdge_k or swdge_qv:
                            nc.gpsimd.dma_start(
                                out=ev.rearrange("p (c x) -> p c x", c=NV),
                                in_=x_sl(NK + NK * NV, C))
                        else:
                            v_t = in_pool.tile([128, NV * ncol], f32,
                                               name=f"v{s}", tag="v",
                                               bufs=qv_bufs)
                            nc.sync.dma_start(
                                out=v_t.rearrange("p (c x) -> p c x", c=NV),
                                in_=x_sl(NK + NK * NV, C))
                            nc.scalar.activation(
                                ev, v_t, mybir.ActivationFunctionType.Copy)
                    else:
                        ev = in_pool.tile([128, NV * ncol], f32, name=f"v{s}",
                                          tag="v", bufs=2)
                        ring = nc.gpsimd if swdge_k else nc.sync
                        ring.dma_start(
                            out=ev.rearrange("p (c x) -> p c x", c=NV),
                            in_=x_sl(NK + NK * NV, C))
                    return ev

                is_last = s + 1 == len(chunk_cols)
                ev_t = None if is_last else emit_v_load()

                acc = work_pool.tile([128, NV * ncol], bf16, name=f"acc{s}",
                                     tag="acc", bufs=acc_bufs)
                acc3 = acc.rearrange("p (v x) -> p v x", v=NV).unsqueeze(1)
                tmp = work_pool.tile([128, NV * ncol], bf16, name=f"tmp{s}",
                                     tag="tmp", bufs=1)
                tmp3 = tmp.rearrange("p (v x) -> p v x", v=NV).unsqueeze(1)

                for k in range(NK):
                    kbf = work_pool.tile([128, NV * ncol], bf16,
                                         name=f"kbf{s}_{k}", tag="kbf",
                                         bufs=kbf_bufs if swdge_k else 2)
                    if swdge_k:
                        nc.gpsimd.dma_start(
                            out=kbf.rearrange("p (c x) -> p c x", c=NV),
                            in_=x_sl(NK + k * NV, NK + (k + 1) * NV))
                    else:
                        kst = in_pool.tile([128, NV * ncol], f32,
                                           name=f"kst{s}_{k}", tag="kst",
                                           bufs=kst_bufs)
                        nc.sync.dma_start(
                            out=kst.rearrange("p (c x) -> p c x", c=NV),
                            in_=x_sl(NK + k * NV, NK + (k + 1) * NV))
                        nc.scalar.activation(kbf, kst,
                                             mybir.ActivationFunctionType.Copy)
                    kbf3 = kbf.rearrange("p (v x) -> p v x", v=NV).unsqueeze(1)
                    q_bk = qbf3[:, k:k + 1].unsqueeze(2).broadcast_to((128, 1, NV, ncol))
                    last_mac = s + 1 == len(chunk_cols) and k == NK - 1
                    if k == 0:
                        nc.vector.tensor_tensor(acc3, q_bk, kbf3, mybir.AluOpType.mult)
                    elif last_mac:
                        # subtile the final multiply-accumulate so the first
                        # exp quarters can start before the full-width add
                        hw = ncol // out_halves
                        for h in range(out_halves):
                            xs = slice(h * hw, (h + 1) * hw)
                            nc.vector.tensor_tensor(
                                tmp3[:, :, :, xs], q_bk[:, :, :, xs],
                                kbf3[:, :, :, xs], mybir.AluOpType.mult)
                            nc.vector.tensor_tensor(
                                acc3[:, :, :, xs], acc3[:, :, :, xs],
                                tmp3[:, :, :, xs], mybir.AluOpType.add)
                    else:
                        nc.vector.tensor_tensor(tmp3, q_bk, kbf3, mybir.AluOpType.mult)
                        nc.vector.tensor_tensor(acc3, acc3, tmp3, mybir.AluOpType.add)
                    if k == 1 and prev is not None:
                        emit_output_phase(*prev, last=False)
                        prev = None
                if is_last:
                    ev_t = emit_v_load()
                flush_stores()
                prev = (s, acc, ev_t, ncol)
            emit_output_phase(*prev, last=True)
    nc.compile()
    return nc


_NC_CACHE = {}

# v1 build configuration (kept as fallback): bf16 product + add-tree,
# ncol=256 chunks with a tapered first/last chunk.  Measured ~161 us/NEFF.
BUILD_CFG = {
    "recip_on_act": False,
    "bf16_tree": True,
    "k_splits": 8,
    "inplace_tree": True,
    "lean_bufs": True,
    "conv_k": True,
    "direct_b": True,
    "chunk_cols": [192, 256, 256, 256, 64],
}

# default build configuration used by kernel(): v2 super-chunk kernel —
# 2KB DMA descriptors ([512, 384, 128] column taper), K streamed in
# 8-channel pieces (ScalarE f32->bf16 downcast, DVE bf16 multiply-
# accumulate), software-pipelined emission (chunk s-1's softmax/output
# runs inside chunk s's K-loop), quarter-subtiled bf16 output phase with
# SWDGE-cast stores, f32 low-latency output path + HWDGE stores on the
# last chunk.  Measured ~151 us/NEFF median on trn2 (8 cores, ~46.1 MB
# traffic/core at ~358 GB/s HBM/core ≈ 129 us DMA floor); rel-l2 vs the
# f32 reference ≈ 5.5e-3 (bf16 qk tree + bf16 softmax/output path).
BUILD_CFG_V2 = {
    "chunk_cols": [512, 384, 128],
    "e_bf16": True,
    "out_halves": 4,
}


def _get_nc(**cfg):
    if cfg.get("version", 2) == 2:
        cfg = {**BUILD_CFG_V2, **{k: v for k, v in cfg.items() if k != "version"}}
        builder = build_nc_v2
    else:
        cfg = {**BUILD_CFG, **{k: v for k, v in cfg.items() if k != "version"}}
        builder = build_nc
    key = tuple(sorted(
        (k, tuple(v) if isinstance(v, list) else v) for k, v in cfg.items()
    )) + (builder.__name__,)
    if key not in _NC_CACHE:
        _NC_CACHE[key] = builder(**cfg)
    return _NC_CACHE[key]


def make_in_maps(inp):
    in_maps = []
    for core in range(N_CORES):
        b, half = core // 2, core % 2
        shard = np.ascontiguousarray(
            inp[b, :, half * ROWS:(half + 1) * ROWS, :], dtype=np.float32
        ).reshape(C, PIX)
        in_maps.append({"x": shard})
    return in_maps


def assemble_out(results):
    out = np.empty((B, NV, H, W), np.float32)
    for core in range(N_CORES):
        b, half = core // 2, core % 2
        out[b, :, half * ROWS:(half + 1) * ROWS, :] = (
            results[core]["y"].reshape(NV, ROWS, W)
        )
    return out


def run_spmd(inp, trace=False, build_cfg=None, **kwargs):
    """Run the SPMD kernel on 8 cores; returns (full_output, BassKernelResults)."""
    _ensure_path()
    from concourse.bass_utils import run_bass_kernel_spmd

    inp = np.asarray(inp)
    assert inp.shape == (B, C, H, W), inp.shape
    nc = _get_nc(**(build_cfg or {}))
    res = run_bass_kernel_spmd(
        nc, make_in_maps(inp), list(range(N_CORES)), trace=trace, **kwargs
    )
    return assemble_out(res.results), res


def kernel(inp):
    out, _ = run_spmd(inp, trace=False)
    return out



# revision 30
# speedup vs baseline: 1.1840x; 1.1840x over previous
"""Trainium2 Bass kernel for per-pixel dot-product attention.

Reference op (per pixel, over C=80 channels split q/k/v = 8/64/8):
    qk[v] = sum_k q[k] * K[k, v] / sqrt(8)
    attn  = softmax(qk over v)
    out[v] = attn[v] * V[v]

Strategy: pure data-parallel over 8 NeuronCores — core i handles batch
i//2, H-rows half (i%2).  ~46.1 MB HBM traffic/core at the ~358 GB/s
per-core HBM limit puts the DMA floor at ~129 us; the kernel is DMA-
bound, so everything is organized around keeping the 16 SDMA engines
streaming:

  * super-chunk pixel layout [512, 384, 128] columns/partition so load
    descriptors are 2KB/1.5KB (≈ HBM line rate; the v1 kernel's 1KB
    descriptors reached only ~91%), with the small last chunk bounding
    the end-of-kernel compute tail;
  * K streamed on the sync HWDGE ring in 8-channel pieces; ScalarE
    downcasts to bf16 and DVE runs the multiply-accumulate at the
    bf16 2x (single-port) rate;
  * software-pipelined emission: chunk s-1's softmax/output phase is
    emitted inside chunk s's K-loop so its ACT exps never head-of-line
    block chunk s's staging conversions, and its stores are issued
    after chunk s's load emission;
  * quarter-subtiled bf16 output phase (exp -> pairwise sums ->
    reciprocal_approx_fast -> *V -> *1/s) with SWDGE dtype-cast stores
    (bf16 SBUF -> f32 HBM), all r handling kept on DVE so the only
    cross-engine handoff per subtile is the exp;
  * the last chunk's output phase runs in f32 with HWDGE stores and a
    column-subtiled final multiply-accumulate to minimize the
    post-last-load critical chain.

Measured ~150 us/NEFF warm median (runs vary ~150-167 us with shared-
device noise) vs 161 us for the v1 kernel; output rel-l2 vs the f32
reference ≈ 5.5e-3 (bf16 qk tree + bf16 softmax/output path).
"""

import numpy as np

NK = 8
NV = 8
C = NK + NK * NV + NV  # 80
B, H, W = 4, 512, 512
N_CORES = 8
ROWS = H // 2            # rows per core
PIX = ROWS * W           # pixels per core (131072)
NCHUNK = 8               # chunks per core
_SCALE = 1.0 / float(np.sqrt(NK))


def _ensure_path():
    import sys
    p = "/opt/trn_rl_repo"
    if p not in sys.path:
        sys.path.insert(0, p)


def build_nc(pix=PIX, nchunk=NCHUNK, recip_on_act=False, bf16_tree=False,
             k_splits=2, inplace_tree=False, lean_bufs=False, conv_k=False,
             chunk_cols=None, split_rings=False, stage_k=False, direct_b=False,
             n_conv_blocks=4):
    """Build the per-core Bass program for a (80, pix) f32 shard.

    All tensor_tensor work runs on DVE (GPSIMD shares an SBUF port with DVE
    and the two engines serialize, so Pool offload is a net loss).  K streams
    in on the sync HWDGE ring in `k_splits` pieces (compute starts after the
    first piece); q/v loads and the output go on the scalar ring.  With
    `inplace_tree` the l1/l2 add-tree levels write back into the prod tile
    (strictly trailing writes, single-engine serial) to fit ncol=256 in SBUF.
    """
    _ensure_path()
    import concourse.tile as tile
    from concourse import bacc, mybir

    f32 = mybir.dt.float32
    mid = mybir.dt.bfloat16 if bf16_tree else f32
    if chunk_cols is None:
        npix = pix // nchunk
        assert npix % 128 == 0
        chunk_cols = [npix // 128] * nchunk
    assert sum(chunk_cols) * 128 == pix

    nc = bacc.Bacc("TRN2", target_bir_lowering=False, debug=False)
    x = nc.dram_tensor("x", [C, pix], f32, kind="ExternalInput")
    y = nc.dram_tensor("y", [NV, pix], f32, kind="ExternalOutput")

    qv_bufs = 1 if lean_bufs else 2
    pipe_bufs = 1 if lean_bufs else 2
    # deferred output DMAs: emit chunk j's store after chunk j+1's input
    # triggers so it never head-of-line-blocks loads on its ring
    pending_out = []

    def flush_out():
        for args in pending_out:
            nc.scalar.dma_start(**args)
        pending_out.clear()

    with tile.TileContext(nc) as tc:
        with (
            tc.tile_pool(name="inp", bufs=2) as in_pool,
            tc.tile_pool(name="work", bufs=1) as work_pool,
            tc.tile_pool(name="pipe", bufs=pipe_bufs) as pipe_pool,
        ):
            off = 0
            for j, ncol in enumerate(chunk_cols):
                npix = 128 * ncol
                q_t = in_pool.tile([128, NK * ncol], f32, name=f"q{j}", tag="q", bufs=qv_bufs)
                v_t = in_pool.tile([128, NV * ncol], f32, name=f"v{j}", tag="v", bufs=qv_bufs)

                # K channel layout is k-major (channel NK + k*NV + v), matching
                # the prod block order; split loads so prod can start early
                sp_ch = NK * NV // k_splits
                if conv_k:  # match the B-half-first compute order
                    dma_order = [h for h in range(k_splits) if h * sp_ch >= NK * NV // 2] + \
                                [h for h in range(k_splits) if h * sp_ch < NK * NV // 2]
                else:
                    dma_order = list(range(k_splits))
                k_stages = {}
                if not stage_k:
                    k_t = in_pool.tile([128, NK * NV * ncol], f32, name=f"k{j}", tag="k")
                    k4 = k_t.rearrange("p (k v x) -> p k v x", k=NK, v=NV)

                def emit_k(h):
                    if stage_k:
                        kst = in_pool.tile([128, sp_ch * ncol], f32,
                                           name=f"kst{j}_{h}", tag="kst", bufs=4)
                        k_stages[h] = kst
                        dst = kst.rearrange("p (c x) -> p c x", c=sp_ch)
                    else:
                        dst = k_t.rearrange("p (c x) -> p c x", c=NK * NV)[
                            :, h * sp_ch:(h + 1) * sp_ch]
                    on_sync = (h * sp_ch >= NK * NV // 2) or not split_rings
                    ring = nc.sync if on_sync else nc.scalar
                    ring.dma_start(
                        out=dst,
                        in_=x[NK + h * sp_ch:NK + (h + 1) * sp_ch, off:off + npix]
                        .rearrange("c (p x) -> p c x", p=128),
                    )

                # ramp: the first K piece goes before q so its descriptors
                # generate first; q loads in halves, upper (k=4..7, feeding
                # the direct B products) first
                emit_k(dma_order[0])
                q_ring = nc.scalar if split_rings else nc.sync
                for cl, ch_ in ((NK // 2, NK), (0, NK // 2)):
                    q_ring.dma_start(
                        out=q_t.rearrange("p (c x) -> p c x", c=NK)[:, cl:ch_],
                        in_=x[cl:ch_, off:off + npix].rearrange("c (p x) -> p c x", p=128),
                    )
                for h in dma_order[1:]:
                    emit_k(h)
                nc.sync.dma_start(
                    out=v_t.rearrange("p (c x) -> p c x", c=NV),
                    in_=x[NK + NK * NV:C, off:off + npix]
                    .rearrange("c (p x) -> p c x", p=128),
                )
                # previous chunk's output, behind this chunk's input triggers
                flush_out()

                # prod[k,v] = q[k] * K[k,v]   (one broadcast multiply per K piece)
                sp_k = NK // k_splits
                if conv_k:
                    # ScalarE downcasts K into the prod tiles and Q into a small
                    # bf16 tile; DVE then multiplies in-place at bf16 2x rate.
                    # prod is split into two half-tiles: B frees after l1, so
                    # the next chunk's conversions overlap this chunk's tail.
                    assert bf16_tree and inplace_tree
                    half = NK // 2
                    prodA = work_pool.tile([128, half * NV * ncol], mid,
                                           name=f"prodA{j}", tag="prodA")
                    prodB = work_pool.tile([128, half * NV * ncol], mid,
                                           name=f"prodB{j}", tag="prodB")
                    p4A = prodA.rearrange("p (k v x) -> p k v x", k=half, v=NV)
                    p4B = prodB.rearrange("p (k v x) -> p k v x", k=half, v=NV)
                    q_bf = work_pool.tile([128, NK * ncol], mybir.dt.bfloat16,
                                          name=f"qbf{j}", tag="qbf")
                    nc.scalar.activation(q_bf, q_t, mybir.ActivationFunctionType.Copy)
                    q_b = (
                        q_bf.rearrange("p (k x) -> p k x", k=NK)
                        .unsqueeze(2)
                        .broadcast_to((128, NK, NV, ncol))
                    )

                    def pslice(kl, kh):  # view of prod blocks [kl, kh)
                        if kh <= half:
                            return p4A[:, kl:kh]
                        assert kl >= half
                        return p4B[:, kl - half:kh - half]

                    # emit B-half first (its tile frees earliest, after l1),
                    # then direct A-blocks, then converted A-blocks — the
                    # direct ones give DVE an ACT-free runway at each boundary
                    ncb = n_conv_blocks
                    a_blocks = [h for h in range(k_splits) if h * sp_k < half]
                    order = [h for h in range(k_splits) if h * sp_k >= half] + \
                            sorted(a_blocks, key=lambda h: h * sp_k < ncb)
                    if direct_b:
                        q_bf32 = (
                            q_t.rearrange("p (k x) -> p k x", k=NK)
                            .unsqueeze(2)
                            .broadcast_to((128, NK, NV, ncol))
                        )
                    for h in order:
                        kl, kh = h * sp_k, (h + 1) * sp_k
                        pv = pslice(kl, kh)
                        if stage_k:
                            src = k_stages[h].rearrange("p (k v x) -> p k v x",
                                                        k=sp_k, v=NV)
                        else:
                            src = k4[:, kl:kh]
                        if direct_b and kl >= ncb:
                            # B half: direct f32 multiply (bf16 out) — no ACT
                            # dependency, so DVE starts as soon as K lands;
                            # ACT meanwhile pre-converts the A half
                            nc.vector.tensor_tensor(
                                pv, q_bf32[:, kl:kh], src, mybir.AluOpType.mult
                            )
                        else:
                            nc.scalar.activation(pv, src,
                                                 mybir.ActivationFunctionType.Copy)
                            nc.vector.tensor_tensor(
                                pv, q_b[:, kl:kh], pv, mybir.AluOpType.mult
                            )
                    # tree: l1 = A + B -> A; l2, qk within A
                    nc.vector.tensor_tensor(p4A, p4A, p4B, mybir.AluOpType.add)
                    l24 = p4A[:, 0:2]
                    nc.vector.tensor_tensor(l24, p4A[:, 0:2], p4A[:, 2:4], mybir.AluOpType.add)
                else:
                    prod = work_pool.tile([128, NK * NV * ncol], mid, name=f"prod{j}", tag="prod")
                    p4 = prod.rearrange("p (k v x) -> p k v x", k=NK, v=NV)
                    q_b = (
                        q_t.rearrange("p (k x) -> p k x", k=NK)
                        .unsqueeze(2)
                        .broadcast_to((128, NK, NV, ncol))
                    )
                    for h in range(k_splits):
                        kl, kh = h * sp_k, (h + 1) * sp_k
                        nc.vector.tensor_tensor(
                            p4[:, kl:kh], q_b[:, kl:kh], k4[:, kl:kh], mybir.AluOpType.mult
                        )

                    # sum over k (outer block index): 3-level pairwise tree (all DVE)
                    if inplace_tree:
                        # l1 -> prod[k 0:4], l2 -> prod[k 0:2]: strictly in-place
                        # (out == in0), serial on DVE
                        l14 = p4[:, 0:4]
                        nc.vector.tensor_tensor(l14, p4[:, 0:4], p4[:, 4:8], mybir.AluOpType.add)
                        l24 = p4[:, 0:2]
                        nc.vector.tensor_tensor(l24, l14[:, 0:2], l14[:, 2:4], mybir.AluOpType.add)
                    else:
                        l1 = work_pool.tile([128, 4 * NV * ncol], mid, name=f"l1_{j}", tag="l1")
                        l14 = l1.rearrange("p (k v x) -> p k v x", k=4, v=NV)
                        nc.vector.tensor_tensor(l14, p4[:, 0:4], p4[:, 4:8], mybir.AluOpType.add)
                        l2 = work_pool.tile([128, 2 * NV * ncol], mid, name=f"l2_{j}", tag="l2")
                        l24 = l2.rearrange("p (k v x) -> p k v x", k=2, v=NV)
                        nc.vector.tensor_tensor(l24, l14[:, 0:2], l14[:, 2:4], mybir.AluOpType.add)
                # qk shares t1's slot: qk dies at exp, t1 is born after exp
                qk = pipe_pool.tile([128, NV * ncol], mid, name=f"qk{j}", tag="t1", bufs=1)
                qk4 = qk.rearrange("p (v x) -> p v x", v=NV).unsqueeze(1)
                nc.vector.tensor_tensor(qk4, l24[:, 0:1], l24[:, 1:2], mybir.AluOpType.add)

                # e = exp(qk / sqrt(NK)); softmax denominators over v
                e_bufs = 1 if max(chunk_cols) > 256 else 2
                e = pipe_pool.tile([128, NV * ncol], f32, name=f"e{j}", tag="e", bufs=e_bufs)
                nc.scalar.activation(e, qk, mybir.ActivationFunctionType.Exp, scale=_SCALE)
                t1 = pipe_pool.tile([128, 4 * ncol], f32, name=f"t1_{j}", tag="t1", bufs=1)
                nc.vector.tensor_tensor(t1, e[:, 0:4 * ncol], e[:, 4 * ncol:], mybir.AluOpType.add)
                # t2 / s / r share one scratch tile (padding control)
                sc = pipe_pool.tile([128, 4 * ncol], f32, name=f"sc{j}", tag="sc", bufs=1)
                t2 = sc[:, 0:2 * ncol]
                nc.vector.tensor_tensor(t2, t1[:, 0:2 * ncol], t1[:, 2 * ncol:], mybir.AluOpType.add)
                s = sc[:, 2 * ncol:3 * ncol]
                nc.vector.tensor_tensor(s, t2[:, 0:ncol], t2[:, ncol:], mybir.AluOpType.add)
                r = sc[:, 3 * ncol:4 * ncol]
                if recip_on_act:
                    # r = exp(-ln s): needs two ACT table sets (thrash) but
                    # stays off the DVE critical path
                    ls = sc[:, 0:ncol]
                    nc.scalar.activation(ls, s, mybir.ActivationFunctionType.Ln)
                    nc.scalar.activation(r, ls, mybir.ActivationFunctionType.Exp, scale=-1.0)
                else:
                    nc.vector.reciprocal(r, s)

                # out[v] = e[v] * V[v] * r  (both multiplies in-place on e; DVE
                # executes them after the t-tree reads of e).  The stride-0
                # broadcast operand goes in in0 — a stride-0 in1 runs at half
                # rate on DVE.
                e3 = e.rearrange("p (v x) -> p v x", v=NV)
                v3 = v_t.rearrange("p (v x) -> p v x", v=NV)
                r_b = r.unsqueeze(1).broadcast_to((128, NV, ncol))
                nc.vector.tensor_tensor(e3, e3, v3, mybir.AluOpType.mult)
                nc.vector.tensor_tensor(e3, r_b, e3, mybir.AluOpType.mult)
                # one output DMA per chunk on the scalar HWDGE ring (deferred)
                pending_out.append(dict(
                    out=y[0:NV, off:off + npix].rearrange("c (p x) -> p c x", p=128),
                    in_=e.rearrange("p (c x) -> p c x", c=NV),
                ))
                off += npix
            flush_out()
    nc.compile()
    return nc


def build_nc_v2(chunk_cols=(512, 512), out_halves=2, kst_bufs=3, e_bufs=1,
                acc_bufs=2, e_bf16=False, defer_stores=False, swdge_k=False,
                kbf_bufs=2, swdge_qv=False, qv_bufs=2, fast_r=True,
                dve_rb=True):
    """v2: super-chunks with 2KB DMA descriptors + streamed-K accumulation.

    Layout per core: (80, 131072) f32 shard, pixels partition-major within
    each super-chunk (partition p owns `ncol` consecutive pixels), so every
    HBM descriptor is ncol*4 bytes (2KB at ncol=512 — full HBM line rate vs
    ~91% at the v1 1KB descriptors).  K streams on the sync ring in 8-channel
    pieces; ScalarE downcasts each piece (and q) to bf16, DVE runs the
    multiply-accumulate at bf16 2x rate.  The softmax/output phase is split
    into `out_halves` column subtiles so the final store pipeline keeps DMA
    busy through the tail.
    """
    _ensure_path()
    import concourse.tile as tile
    from concourse import bacc, mybir

    f32 = mybir.dt.float32
    bf16 = mybir.dt.bfloat16
    edt = bf16 if e_bf16 else f32
    chunk_cols = list(chunk_cols)
    assert sum(chunk_cols) * 128 == PIX

    nc = bacc.Bacc("TRN2", target_bir_lowering=False, debug=False)
    x = nc.dram_tensor("x", [C, PIX], f32, kind="ExternalInput")
    y = nc.dram_tensor("y", [NV, PIX], f32, kind="ExternalOutput")

    with tile.TileContext(nc) as tc:
        with (
            tc.tile_pool(name="inp", bufs=2) as in_pool,
            tc.tile_pool(name="work", bufs=1) as work_pool,
        ):
            offs = []
            o = 0
            for ncol in chunk_cols:
                offs.append(o)
                o += 128 * ncol

            pending_stores = []

            def flush_stores():
                for args in pending_stores:
                    if e_bf16:
                        nc.gpsimd.dma_start(**args)  # SWDGE casts bf16 -> f32
                    else:
                        nc.scalar.dma_start(**args)
                pending_stores.clear()

            def emit_output_phase(s, acc, v_t, ncol, last):
                """softmax + output for chunk s, subtiled over columns.

                Big chunks run the e-path in bf16 (DVE 2x) with SWDGE-cast
                stores.  The last (small, latency-critical) chunk runs it in
                f32: at tiny FD the op costs are similar and f32 drops the
                rbf ACT hop and the SWDGE dispatch/completion latency from
                the end-of-kernel critical chain (HWDGE store, no cast).
                """
                cdt = f32 if last else edt
                off, npix = offs[s], 128 * ncol
                acc_v = acc.rearrange("p (v x) -> p v x", v=NV)
                e = work_pool.tile([128, NV * ncol], cdt, name=f"e{s}",
                                   tag="e" if cdt is edt else "ef", bufs=e_bufs)
                e_v = e.rearrange("p (v x) -> p v x", v=NV)
                v3 = v_t.rearrange("p (c x) -> p c x", c=NV)
                hw = ncol // out_halves
                for h in range(out_halves):
                    lo, hi = h * hw, (h + 1) * hw
                    e_h = e_v[:, :, lo:hi]
                    nc.scalar.activation(e_h, acc_v[:, :, lo:hi],
                                         mybir.ActivationFunctionType.Exp,
                                         scale=_SCALE)
                    t1 = work_pool.tile([128, 4 * hw], cdt, name=f"t1_{s}{h}",
                                        tag="t1", bufs=1)
                    t13 = t1.rearrange("p (c x) -> p c x", c=4)
                    nc.vector.tensor_tensor(t13, e_v[:, 0:4, lo:hi],
                                            e_v[:, 4:8, lo:hi], mybir.AluOpType.add)
                    t2t = work_pool.tile([128, 2 * hw], cdt, name=f"t2_{s}{h}",
                                         tag="t2", bufs=1)
                    t2 = t2t.rearrange("p (c x) -> p c x", c=2)
                    nc.vector.tensor_tensor(t2, t13[:, 0:2], t13[:, 2:4],
                                            mybir.AluOpType.add)
                    sc4 = work_pool.tile([128, 3 * hw], f32, name=f"sc{s}{h}",
                                         tag="sc", bufs=1)
                    ssum = sc4[:, 0:hw]
                    nc.vector.tensor_tensor(ssum, t2[:, 0], t2[:, 1],
                                            mybir.AluOpType.add)
                    r = sc4[:, hw:2 * hw]
                    if fast_r:
                        # s = sum of 8 exp() values, strictly positive and in
                        # range, so the fast approx (18 bits ≫ bf16) is safe
                        nc.vector.reciprocal_approx_fast(r, ssum)
                    else:
                        nc.vector.reciprocal(r, ssum)
                    if cdt is not f32:
                        rb_t = work_pool.tile([128, hw], edt, name=f"rb{s}{h}",
                                              tag="rb", bufs=1)
                        if dve_rb:
                            # DVE-side cast: keeps the r hop off ACT so the
                            # only cross-engine handoff per subtile is the exp
                            nc.vector.tensor_scalar_mul(rb_t, r, 1.0)
                        else:
                            nc.scalar.activation(
                                rb_t, r, mybir.ActivationFunctionType.Copy)
                        r_b = rb_t.unsqueeze(1).broadcast_to((128, NV, hw))
                    else:
                        r_b = r.unsqueeze(1).broadcast_to((128, NV, hw))
                    nc.vector.tensor_tensor(e_h, e_h, v3[:, :, lo:hi],
                                            mybir.AluOpType.mult)
                    nc.vector.tensor_tensor(e_h, r_b, e_h, mybir.AluOpType.mult)
                    st = dict(
                        out=y[0:NV, off:off + npix]
                        .rearrange("c (p x) -> p c x", p=128)[:, :, lo:hi],
                        in_=e_h,
                    )
                    if last:
                        nc.scalar.dma_start(**st)
                    else:
                        # issued after the NEXT chunk's load emission so the
                        # store's wait-on-DVE never head-of-line-blocks the
                        # load descriptor stream on its engine
                        pending_stores.append(st)

            # software-pipelined emission: chunk s's softmax/output phase is
            # emitted at the end of chunk s, so its DVE chain runs in the DVE
            # idle window while chunk s+1's q/v/K0 load, instead of competing
            # with chunk s+1's multiply-accumulates.  Its stores are deferred
            # to after chunk s+1's load emission (flush inside the K-loop) so
            # their wait-on-DVE never head-of-line-blocks a load engine; exp
            # is the only ACT op in the chain, and it runs while ACT waits
            # for chunk s+1's q to land.
            for s, ncol in enumerate(chunk_cols):
                off, npix = offs[s], 128 * ncol
                x_sl = lambda c0, c1: (
                    x[c0:c1, off:off + npix].rearrange("c (p x) -> p c x", p=128))
                # ---- q
                q_bf = work_pool.tile([128, NK * ncol], bf16, name=f"qbf{s}",
                                      tag="qbf", bufs=2)
                if swdge_k or swdge_qv:
                    nc.gpsimd.dma_start(
                        out=q_bf.rearrange("p (c x) -> p c x", c=NK),
                        in_=x_sl(0, NK))
                else:
                    q_t = in_pool.tile([128, NK * ncol], f32, name=f"q{s}",
                                       tag="q", bufs=qv_bufs)
                    nc.sync.dma_start(
                        out=q_t.rearrange("p (c x) -> p c x", c=NK), in_=x_sl(0, NK))
                    nc.scalar.activation(q_bf, q_t,
                                         mybir.ActivationFunctionType.Copy)
                qbf3 = q_bf.rearrange("p (k x) -> p k x", k=NK)

                def emit_v_load():
                    # v is consumed only at e*v in the NEXT chunk's window, so
                    # it loads after this chunk's K pieces — for the last chunk
                    # that lets the matvec+softmax chain run during v's
                    # transfer instead of after it.
                    if e_bf16 and s + 1 < len(chunk_cols):
                        ev = work_pool.tile([128, NV * ncol], bf16,
                                            name=f"vbf{s}", tag="vbf", bufs=2)
                        if swdge_k or swdge_qv:
                            nc.gpsimd.dma_start(
                                out=ev.rearrange("p (c x) -> p c x", c=NV),
                                in_=x_sl(NK + NK * NV, C))
                        else:
                            v_t = in_pool.tile([128, NV * ncol], f32,
                                               name=f"v{s}", tag="v",
                                               bufs=qv_bufs)
                            nc.sync.dma_start(
                                out=v_t.rearrange("p (c x) -> p c x", c=NV),
                                in_=x_sl(NK + NK * NV, C))
                            nc.scalar.activation(
                                ev, v_t, mybir.ActivationFunctionType.Copy)
                    else:
                        ev = in_pool.tile([128, NV * ncol], f32, name=f"v{s}",
                                          tag="v", bufs=qv_bufs)
                        ring = nc.gpsimd if swdge_k else nc.sync
                        ring.dma_start(
                            out=ev.rearrange("p (c x) -> p c x", c=NV),
                            in_=x_sl(NK + NK * NV, C))
                    return ev

                is_last = s + 1 == len(chunk_cols)
                ev_t = None if is_last else emit_v_load()

                acc = work_pool.tile([128, NV * ncol], bf16, name=f"acc{s}",
                                     tag="acc", bufs=acc_bufs)
                acc3 = acc.rearrange("p (v x) -> p v x", v=NV).unsqueeze(1)
                tmp = work_pool.tile([128, NV * ncol], bf16, name=f"tmp{s}",
                                     tag="tmp", bufs=1)
                tmp3 = tmp.rearrange("p (v x) -> p v x", v=NV).unsqueeze(1)

                for k in range(NK):
                    kbf = work_pool.tile([128, NV * ncol], bf16,
                                         name=f"kbf{s}_{k}", tag="kbf",
                                         bufs=kbf_bufs if swdge_k else 2)
                    if swdge_k:
                        nc.gpsimd.dma_start(
                            out=kbf.rearrange("p (c x) -> p c x", c=NV),
                            in_=x_sl(NK + k * NV, NK + (k + 1) * NV))
                    else:
                        kst = in_pool.tile([128, NV * ncol], f32,
                                           name=f"kst{s}_{k}", tag="kst",
                                           bufs=kst_bufs)
                        nc.sync.dma_start(
                            out=kst.rearrange("p (c x) -> p c x", c=NV),
                            in_=x_sl(NK + k * NV, NK + (k + 1) * NV))
                        nc.scalar.activation(kbf, kst,
                                             mybir.ActivationFunctionType.Copy)
                    kbf3 = kbf.rearrange("p (v x) -> p v x", v=NV).unsqueeze(1)
                    q_bk = qbf3[:, k:k + 1].unsqueeze(2).broadcast_to((128, 1, NV, ncol))
                    last_mac = s + 1 == len(chunk_cols) and k == NK - 1
                    if k == 0:
                        nc.vector.tensor_tensor(acc3, q_bk, kbf3, mybir.AluOpType.mult)
                    elif last_mac:
                        # subtile the final multiply-accumulate so the first
                        # exp quarters can start before the full-width add
                        hw = ncol // out_halves
                        for h in range(out_halves):
                            xs = slice(h * hw, (h + 1) * hw)
                            nc.vector.tensor_tensor(
                                tmp3[:, :, :, xs], q_bk[:, :, :, xs],
                                kbf3[:, :, :, xs], mybir.AluOpType.mult)
                            nc.vector.tensor_tensor(
                                acc3[:, :, :, xs], acc3[:, :, :, xs],
                                tmp3[:, :, :, xs], mybir.AluOpType.add)
                    else:
                        nc.vector.tensor_tensor(tmp3, q_bk, kbf3, mybir.AluOpType.mult)
                        nc.vector.tensor_tensor(acc3, acc3, tmp3, mybir.AluOpType.add)
                    if k == 1:
                        # previous chunk's stores go out behind this chunk's
                        # first loads
                        flush_stores()
                if is_last:
                    ev_t = emit_v_load()
                emit_output_phase(s, acc, ev_t, ncol, last=is_last)
    nc.compile()
    return nc


_NC_CACHE = {}

# v1 build configuration (kept as fallback): bf16 product + add-tree,
# ncol=256 chunks with a tapered first/last chunk.  Measured ~161 us/NEFF.
BUILD_CFG = {
    "recip_on_act": False,
    "bf16_tree": True,
    "k_splits": 8,
    "inplace_tree": True,
    "lean_bufs": True,
    "conv_k": True,
    "direct_b": True,
    "chunk_cols": [192, 256, 256, 256, 64],
}

# default build configuration used by kernel(): v2 super-chunk kernel —
# 2KB DMA descriptors ([512, 384, 128] column taper), K streamed in
# 8-channel pieces (ScalarE f32->bf16 downcast, DVE bf16 multiply-
# accumulate), software-pipelined emission (chunk s-1's softmax/output
# runs inside chunk s's K-loop), quarter-subtiled bf16 output phase with
# SWDGE-cast stores, f32 low-latency output path + HWDGE stores on the
# last chunk.  Measured ~151 us/NEFF median on trn2 (8 cores, ~46.1 MB
# traffic/core at ~358 GB/s HBM/core ≈ 129 us DMA floor); rel-l2 vs the
# f32 reference ≈ 5.5e-3 (bf16 qk tree + bf16 softmax/output path).
BUILD_CFG_V2 = {
    "chunk_cols": [512, 384, 128],
    "e_bf16": True,
    "out_halves": 4,
}


def _get_nc(**cfg):
    if cfg.get("version", 2) == 2:
        cfg = {**BUILD_CFG_V2, **{k: v for k, v in cfg.items() if k != "version"}}
        builder = build_nc_v2
    else:
        cfg = {**BUILD_CFG, **{k: v for k, v in cfg.items() if k != "version"}}
        builder = build_nc
    key = tuple(sorted(
        (k, tuple(v) if isinstance(v, list) else v) for k, v in cfg.items()
    )) + (builder.__name__,)
    if key not in _NC_CACHE:
        _NC_CACHE[key] = builder(**cfg)
    return _NC_CACHE[key]


def make_in_maps(inp):
    in_maps = []
    for core in range(N_CORES):
        b, half = core // 2, core % 2
        shard = np.ascontiguousarray(
            inp[b, :, half * ROWS:(half + 1) * ROWS, :], dtype=np.float32
        ).reshape(C, PIX)
        in_maps.append({"x": shard})
    return in_maps


def assemble_out(results):
    out = np.empty((B, NV, H, W), np.float32)
    for core in range(N_CORES):
        b, half = core // 2, core % 2
        out[b, :, half * ROWS:(half + 1) * ROWS, :] = (
            results[core]["y"].reshape(NV, ROWS, W)
        )
    return out


def run_spmd(inp, trace=False, build_cfg=None, **kwargs):
    """Run the SPMD kernel on 8 cores; returns (full_output, BassKernelResults)."""
    _ensure_path()
    from concourse.bass_utils import run_bass_kernel_spmd

    inp = np.asarray(inp)
    assert inp.shape == (B, C, H, W), inp.shape
    nc = _get_nc(**(build_cfg or {}))
    res = run_bass_kernel_spmd(
        nc, make_in_maps(inp), list(range(N_CORES)), trace=trace, **kwargs
    )
    return assemble_out(res.results), res


def kernel(inp):
    out, _ = run_spmd(inp, trace=False)
    return out



# revision 32
# speedup vs baseline: 1.2150x; 1.0262x over previous
"""Trainium2 Bass kernel for per-pixel dot-product attention.

Reference op (per pixel, over C=80 channels split q/k/v = 8/64/8):
    qk[v] = sum_k q[k] * K[k, v] / sqrt(8)
    attn  = softmax(qk over v)
    out[v] = attn[v] * V[v]

Strategy: pure data-parallel over 8 NeuronCores — core i handles batch
i//2, H-rows half (i%2).  ~46.1 MB HBM traffic/core at the ~358 GB/s
per-core HBM limit puts the DMA floor at ~129 us; the kernel is DMA-
bound, so everything is organized around keeping the 16 SDMA engines
streaming:

  * super-chunk pixel layout [512, 384, 128] columns/partition so load
    descriptors are 2KB/1.5KB (≈ HBM line rate; the v1 kernel's 1KB
    descriptors reached only ~91%), with the small last chunk bounding
    the end-of-kernel compute tail;
  * K streamed on the sync HWDGE ring in 8-channel pieces; ScalarE
    downcasts to bf16 and DVE runs the multiply-accumulate at the
    bf16 2x (single-port) rate;
  * software-pipelined emission: chunk s-1's softmax/output phase is
    emitted inside chunk s's K-loop so its ACT exps never head-of-line
    block chunk s's staging conversions, and its stores are issued
    after chunk s's load emission;
  * quarter-subtiled bf16 output phase (exp -> pairwise sums ->
    reciprocal_approx_fast -> *V -> *1/s) with SWDGE dtype-cast stores
    (bf16 SBUF -> f32 HBM), all r handling kept on DVE so the only
    cross-engine handoff per subtile is the exp;
  * the last chunk's output phase runs in f32 with HWDGE stores and a
    column-subtiled final multiply-accumulate to minimize the
    post-last-load critical chain.

Measured ~150 us/NEFF warm median (runs vary ~150-167 us with shared-
device noise) vs 161 us for the v1 kernel; output rel-l2 vs the f32
reference ≈ 5.5e-3 (bf16 qk tree + bf16 softmax/output path).
"""

import numpy as np

NK = 8
NV = 8
C = NK + NK * NV + NV  # 80
B, H, W = 4, 512, 512
N_CORES = 8
ROWS = H // 2            # rows per core
PIX = ROWS * W           # pixels per core (131072)
NCHUNK = 8               # chunks per core
_SCALE = 1.0 / float(np.sqrt(NK))


def _ensure_path():
    import sys
    p = "/opt/trn_rl_repo"
    if p not in sys.path:
        sys.path.insert(0, p)


def build_nc(pix=PIX, nchunk=NCHUNK, recip_on_act=False, bf16_tree=False,
             k_splits=2, inplace_tree=False, lean_bufs=False, conv_k=False,
             chunk_cols=None, split_rings=False, stage_k=False, direct_b=False,
             n_conv_blocks=4):
    """Build the per-core Bass program for a (80, pix) f32 shard.

    All tensor_tensor work runs on DVE (GPSIMD shares an SBUF port with DVE
    and the two engines serialize, so Pool offload is a net loss).  K streams
    in on the sync HWDGE ring in `k_splits` pieces (compute starts after the
    first piece); q/v loads and the output go on the scalar ring.  With
    `inplace_tree` the l1/l2 add-tree levels write back into the prod tile
    (strictly trailing writes, single-engine serial) to fit ncol=256 in SBUF.
    """
    _ensure_path()
    import concourse.tile as tile
    from concourse import bacc, mybir

    f32 = mybir.dt.float32
    mid = mybir.dt.bfloat16 if bf16_tree else f32
    if chunk_cols is None:
        npix = pix // nchunk
        assert npix % 128 == 0
        chunk_cols = [npix // 128] * nchunk
    assert sum(chunk_cols) * 128 == pix

    nc = bacc.Bacc("TRN2", target_bir_lowering=False, debug=False)
    x = nc.dram_tensor("x", [C, pix], f32, kind="ExternalInput")
    y = nc.dram_tensor("y", [NV, pix], f32, kind="ExternalOutput")

    qv_bufs = 1 if lean_bufs else 2
    pipe_bufs = 1 if lean_bufs else 2
    # deferred output DMAs: emit chunk j's store after chunk j+1's input
    # triggers so it never head-of-line-blocks loads on its ring
    pending_out = []

    def flush_out():
        for args in pending_out:
            nc.scalar.dma_start(**args)
        pending_out.clear()

    with tile.TileContext(nc) as tc:
        with (
            tc.tile_pool(name="inp", bufs=2) as in_pool,
            tc.tile_pool(name="work", bufs=1) as work_pool,
            tc.tile_pool(name="pipe", bufs=pipe_bufs) as pipe_pool,
        ):
            off = 0
            for j, ncol in enumerate(chunk_cols):
                npix = 128 * ncol
                q_t = in_pool.tile([128, NK * ncol], f32, name=f"q{j}", tag="q", bufs=qv_bufs)
                v_t = in_pool.tile([128, NV * ncol], f32, name=f"v{j}", tag="v", bufs=qv_bufs)

                # K channel layout is k-major (channel NK + k*NV + v), matching
                # the prod block order; split loads so prod can start early
                sp_ch = NK * NV // k_splits
                if conv_k:  # match the B-half-first compute order
                    dma_order = [h for h in range(k_splits) if h * sp_ch >= NK * NV // 2] + \
                                [h for h in range(k_splits) if h * sp_ch < NK * NV // 2]
                else:
                    dma_order = list(range(k_splits))
                k_stages = {}
                if not stage_k:
                    k_t = in_pool.tile([128, NK * NV * ncol], f32, name=f"k{j}", tag="k")
                    k4 = k_t.rearrange("p (k v x) -> p k v x", k=NK, v=NV)

                def emit_k(h):
                    if stage_k:
                        kst = in_pool.tile([128, sp_ch * ncol], f32,
                                           name=f"kst{j}_{h}", tag="kst", bufs=4)
                        k_stages[h] = kst
                        dst = kst.rearrange("p (c x) -> p c x", c=sp_ch)
                    else:
                        dst = k_t.rearrange("p (c x) -> p c x", c=NK * NV)[
                            :, h * sp_ch:(h + 1) * sp_ch]
                    on_sync = (h * sp_ch >= NK * NV // 2) or not split_rings
                    ring = nc.sync if on_sync else nc.scalar
                    ring.dma_start(
                        out=dst,
                        in_=x[NK + h * sp_ch:NK + (h + 1) * sp_ch, off:off + npix]
                        .rearrange("c (p x) -> p c x", p=128),
                    )

                # ramp: the first K piece goes before q so its descriptors
                # generate first; q loads in halves, upper (k=4..7, feeding
                # the direct B products) first
                emit_k(dma_order[0])
                q_ring = nc.scalar if split_rings else nc.sync
                for cl, ch_ in ((NK // 2, NK), (0, NK // 2)):
                    q_ring.dma_start(
                        out=q_t.rearrange("p (c x) -> p c x", c=NK)[:, cl:ch_],
                        in_=x[cl:ch_, off:off + npix].rearrange("c (p x) -> p c x", p=128),
                    )
                for h in dma_order[1:]:
                    emit_k(h)
                nc.sync.dma_start(
                    out=v_t.rearrange("p (c x) -> p c x", c=NV),
                    in_=x[NK + NK * NV:C, off:off + npix]
                    .rearrange("c (p x) -> p c x", p=128),
                )
                # previous chunk's output, behind this chunk's input triggers
                flush_out()

                # prod[k,v] = q[k] * K[k,v]   (one broadcast multiply per K piece)
                sp_k = NK // k_splits
                if conv_k:
                    # ScalarE downcasts K into the prod tiles and Q into a small
                    # bf16 tile; DVE then multiplies in-place at bf16 2x rate.
                    # prod is split into two half-tiles: B frees after l1, so
                    # the next chunk's conversions overlap this chunk's tail.
                    assert bf16_tree and inplace_tree
                    half = NK // 2
                    prodA = work_pool.tile([128, half * NV * ncol], mid,
                                           name=f"prodA{j}", tag="prodA")
                    prodB = work_pool.tile([128, half * NV * ncol], mid,
                                           name=f"prodB{j}", tag="prodB")
                    p4A = prodA.rearrange("p (k v x) -> p k v x", k=half, v=NV)
                    p4B = prodB.rearrange("p (k v x) -> p k v x", k=half, v=NV)
                    q_bf = work_pool.tile([128, NK * ncol], mybir.dt.bfloat16,
                                          name=f"qbf{j}", tag="qbf")
                    nc.scalar.activation(q_bf, q_t, mybir.ActivationFunctionType.Copy)
                    q_b = (
                        q_bf.rearrange("p (k x) -> p k x", k=NK)
                        .unsqueeze(2)
                        .broadcast_to((128, NK, NV, ncol))
                    )

                    def pslice(kl, kh):  # view of prod blocks [kl, kh)
                        if kh <= half:
                            return p4A[:, kl:kh]
                        assert kl >= half
                        return p4B[:, kl - half:kh - half]

                    # emit B-half first (its tile frees earliest, after l1),
                    # then direct A-blocks, then converted A-blocks — the
                    # direct ones give DVE an ACT-free runway at each boundary
                    ncb = n_conv_blocks
                    a_blocks = [h for h in range(k_splits) if h * sp_k < half]
                    order = [h for h in range(k_splits) if h * sp_k >= half] + \
                            sorted(a_blocks, key=lambda h: h * sp_k < ncb)
                    if direct_b:
                        q_bf32 = (
                            q_t.rearrange("p (k x) -> p k x", k=NK)
                            .unsqueeze(2)
                            .broadcast_to((128, NK, NV, ncol))
                        )
                    for h in order:
                        kl, kh = h * sp_k, (h + 1) * sp_k
                        pv = pslice(kl, kh)
                        if stage_k:
                            src = k_stages[h].rearrange("p (k v x) -> p k v x",
                                                        k=sp_k, v=NV)
                        else:
                            src = k4[:, kl:kh]
                        if direct_b and kl >= ncb:
                            # B half: direct f32 multiply (bf16 out) — no ACT
                            # dependency, so DVE starts as soon as K lands;
                            # ACT meanwhile pre-converts the A half
                            nc.vector.tensor_tensor(
                                pv, q_bf32[:, kl:kh], src, mybir.AluOpType.mult
                            )
                        else:
                            nc.scalar.activation(pv, src,
                                                 mybir.ActivationFunctionType.Copy)
                            nc.vector.tensor_tensor(
                                pv, q_b[:, kl:kh], pv, mybir.AluOpType.mult
                            )
                    # tree: l1 = A + B -> A; l2, qk within A
                    nc.vector.tensor_tensor(p4A, p4A, p4B, mybir.AluOpType.add)
                    l24 = p4A[:, 0:2]
                    nc.vector.tensor_tensor(l24, p4A[:, 0:2], p4A[:, 2:4], mybir.AluOpType.add)
                else:
                    prod = work_pool.tile([128, NK * NV * ncol], mid, name=f"prod{j}", tag="prod")
                    p4 = prod.rearrange("p (k v x) -> p k v x", k=NK, v=NV)
                    q_b = (
                        q_t.rearrange("p (k x) -> p k x", k=NK)
                        .unsqueeze(2)
                        .broadcast_to((128, NK, NV, ncol))
                    )
                    for h in range(k_splits):
                        kl, kh = h * sp_k, (h + 1) * sp_k
                        nc.vector.tensor_tensor(
                            p4[:, kl:kh], q_b[:, kl:kh], k4[:, kl:kh], mybir.AluOpType.mult
                        )

                    # sum over k (outer block index): 3-level pairwise tree (all DVE)
                    if inplace_tree:
                        # l1 -> prod[k 0:4], l2 -> prod[k 0:2]: strictly in-place
                        # (out == in0), serial on DVE
                        l14 = p4[:, 0:4]
                        nc.vector.tensor_tensor(l14, p4[:, 0:4], p4[:, 4:8], mybir.AluOpType.add)
                        l24 = p4[:, 0:2]
                        nc.vector.tensor_tensor(l24, l14[:, 0:2], l14[:, 2:4], mybir.AluOpType.add)
                    else:
                        l1 = work_pool.tile([128, 4 * NV * ncol], mid, name=f"l1_{j}", tag="l1")
                        l14 = l1.rearrange("p (k v x) -> p k v x", k=4, v=NV)
                        nc.vector.tensor_tensor(l14, p4[:, 0:4], p4[:, 4:8], mybir.AluOpType.add)
                        l2 = work_pool.tile([128, 2 * NV * ncol], mid, name=f"l2_{j}", tag="l2")
                        l24 = l2.rearrange("p (k v x) -> p k v x", k=2, v=NV)
                        nc.vector.tensor_tensor(l24, l14[:, 0:2], l14[:, 2:4], mybir.AluOpType.add)
                # qk shares t1's slot: qk dies at exp, t1 is born after exp
                qk = pipe_pool.tile([128, NV * ncol], mid, name=f"qk{j}", tag="t1", bufs=1)
                qk4 = qk.rearrange("p (v x) -> p v x", v=NV).unsqueeze(1)
                nc.vector.tensor_tensor(qk4, l24[:, 0:1], l24[:, 1:2], mybir.AluOpType.add)

                # e = exp(qk / sqrt(NK)); softmax denominators over v
                e_bufs = 1 if max(chunk_cols) > 256 else 2
                e = pipe_pool.tile([128, NV * ncol], f32, name=f"e{j}", tag="e", bufs=e_bufs)
                nc.scalar.activation(e, qk, mybir.ActivationFunctionType.Exp, scale=_SCALE)
                t1 = pipe_pool.tile([128, 4 * ncol], f32, name=f"t1_{j}", tag="t1", bufs=1)
                nc.vector.tensor_tensor(t1, e[:, 0:4 * ncol], e[:, 4 * ncol:], mybir.AluOpType.add)
                # t2 / s / r share one scratch tile (padding control)
                sc = pipe_pool.tile([128, 4 * ncol], f32, name=f"sc{j}", tag="sc", bufs=1)
                t2 = sc[:, 0:2 * ncol]
                nc.vector.tensor_tensor(t2, t1[:, 0:2 * ncol], t1[:, 2 * ncol:], mybir.AluOpType.add)
                s = sc[:, 2 * ncol:3 * ncol]
                nc.vector.tensor_tensor(s, t2[:, 0:ncol], t2[:, ncol:], mybir.AluOpType.add)
                r = sc[:, 3 * ncol:4 * ncol]
                if recip_on_act:
                    # r = exp(-ln s): needs two ACT table sets (thrash) but
                    # stays off the DVE critical path
                    ls = sc[:, 0:ncol]
                    nc.scalar.activation(ls, s, mybir.ActivationFunctionType.Ln)
                    nc.scalar.activation(r, ls, mybir.ActivationFunctionType.Exp, scale=-1.0)
                else:
                    nc.vector.reciprocal(r, s)

                # out[v] = e[v] * V[v] * r  (both multiplies in-place on e; DVE
                # executes them after the t-tree reads of e).  The stride-0
                # broadcast operand goes in in0 — a stride-0 in1 runs at half
                # rate on DVE.
                e3 = e.rearrange("p (v x) -> p v x", v=NV)
                v3 = v_t.rearrange("p (v x) -> p v x", v=NV)
                r_b = r.unsqueeze(1).broadcast_to((128, NV, ncol))
                nc.vector.tensor_tensor(e3, e3, v3, mybir.AluOpType.mult)
                nc.vector.tensor_tensor(e3, r_b, e3, mybir.AluOpType.mult)
                # one output DMA per chunk on the scalar HWDGE ring (deferred)
                pending_out.append(dict(
                    out=y[0:NV, off:off + npix].rearrange("c (p x) -> p c x", p=128),
                    in_=e.rearrange("p (c x) -> p c x", c=NV),
                ))
                off += npix
            flush_out()
    nc.compile()
    return nc


def build_nc_v2(chunk_cols=(512, 512), out_halves=2, kst_bufs=3, e_bufs=1,
                acc_bufs=2, e_bf16=False, defer_stores=False, swdge_k=False,
                kbf_bufs=2, swdge_qv=False, qv_bufs=2, fast_r=True,
                dve_rb=True):
    """v2: super-chunks with 2KB DMA descriptors + streamed-K accumulation.

    Layout per core: (80, 131072) f32 shard, pixels partition-major within
    each super-chunk (partition p owns `ncol` consecutive pixels), so every
    HBM descriptor is ncol*4 bytes (2KB at ncol=512 — full HBM line rate vs
    ~91% at the v1 1KB descriptors).  K streams on the sync ring in 8-channel
    pieces; ScalarE downcasts each piece (and q) to bf16, DVE runs the
    multiply-accumulate at bf16 2x rate.  The softmax/output phase is split
    into `out_halves` column subtiles so the final store pipeline keeps DMA
    busy through the tail.
    """
    _ensure_path()
    import concourse.tile as tile
    from concourse import bacc, mybir

    f32 = mybir.dt.float32
    bf16 = mybir.dt.bfloat16
    edt = bf16 if e_bf16 else f32
    chunk_cols = list(chunk_cols)
    assert sum(chunk_cols) * 128 == PIX

    nc = bacc.Bacc("TRN2", target_bir_lowering=False, debug=False)
    x = nc.dram_tensor("x", [C, PIX], f32, kind="ExternalInput")
    y = nc.dram_tensor("y", [NV, PIX], f32, kind="ExternalOutput")

    with tile.TileContext(nc) as tc:
        with (
            tc.tile_pool(name="inp", bufs=2) as in_pool,
            tc.tile_pool(name="work", bufs=1) as work_pool,
        ):
            offs = []
            o = 0
            for ncol in chunk_cols:
                offs.append(o)
                o += 128 * ncol

            pending_stores = []

            def flush_stores():
                for args in pending_stores:
                    if e_bf16:
                        nc.gpsimd.dma_start(**args)  # SWDGE casts bf16 -> f32
                    else:
                        nc.scalar.dma_start(**args)
                pending_stores.clear()

            def emit_output_phase(s, acc, v_t, ncol, last):
                """softmax + output for chunk s, subtiled over columns.

                Big chunks run the e-path in bf16 (DVE 2x) with SWDGE-cast
                stores.  The last (small, latency-critical) chunk runs it in
                f32: at tiny FD the op costs are similar and f32 drops the
                rbf ACT hop and the SWDGE dispatch/completion latency from
                the end-of-kernel critical chain (HWDGE store, no cast).
                """
                cdt = f32 if last else edt
                off, npix = offs[s], 128 * ncol
                acc_v = acc.rearrange("p (v x) -> p v x", v=NV)
                e = work_pool.tile([128, NV * ncol], cdt, name=f"e{s}",
                                   tag="e" if cdt is edt else "ef", bufs=e_bufs)
                e_v = e.rearrange("p (v x) -> p v x", v=NV)
                v3 = v_t.rearrange("p (c x) -> p c x", c=NV)
                oh = max(1, min(out_halves, ncol // 32))
                hw = ncol // oh
                for h in range(oh):
                    lo, hi = h * hw, (h + 1) * hw
                    e_h = e_v[:, :, lo:hi]
                    nc.scalar.activation(e_h, acc_v[:, :, lo:hi],
                                         mybir.ActivationFunctionType.Exp,
                                         scale=_SCALE)
                    t1 = work_pool.tile([128, 4 * hw], cdt, name=f"t1_{s}{h}",
                                        tag="t1", bufs=1)
                    t13 = t1.rearrange("p (c x) -> p c x", c=4)
                    nc.vector.tensor_tensor(t13, e_v[:, 0:4, lo:hi],
                                            e_v[:, 4:8, lo:hi], mybir.AluOpType.add)
                    t2t = work_pool.tile([128, 2 * hw], cdt, name=f"t2_{s}{h}",
                                         tag="t2", bufs=1)
                    t2 = t2t.rearrange("p (c x) -> p c x", c=2)
                    nc.vector.tensor_tensor(t2, t13[:, 0:2], t13[:, 2:4],
                                            mybir.AluOpType.add)
                    sc4 = work_pool.tile([128, 3 * hw], f32, name=f"sc{s}{h}",
                                         tag="sc", bufs=1)
                    ssum = sc4[:, 0:hw]
                    nc.vector.tensor_tensor(ssum, t2[:, 0], t2[:, 1],
                                            mybir.AluOpType.add)
                    r = sc4[:, hw:2 * hw]
                    if fast_r:
                        # s = sum of 8 exp() values, strictly positive and in
                        # range, so the fast approx (18 bits ≫ bf16) is safe
                        nc.vector.reciprocal_approx_fast(r, ssum)
                    else:
                        nc.vector.reciprocal(r, ssum)
                    if cdt is not f32:
                        rb_t = work_pool.tile([128, hw], edt, name=f"rb{s}{h}",
                                              tag="rb", bufs=1)
                        if dve_rb:
                            # DVE-side cast: keeps the r hop off ACT so the
                            # only cross-engine handoff per subtile is the exp
                            nc.vector.tensor_scalar_mul(rb_t, r, 1.0)
                        else:
                            nc.scalar.activation(
                                rb_t, r, mybir.ActivationFunctionType.Copy)
                        r_b = rb_t.unsqueeze(1).broadcast_to((128, NV, hw))
                    else:
                        r_b = r.unsqueeze(1).broadcast_to((128, NV, hw))
                    nc.vector.tensor_tensor(e_h, e_h, v3[:, :, lo:hi],
                                            mybir.AluOpType.mult)
                    nc.vector.tensor_tensor(e_h, r_b, e_h, mybir.AluOpType.mult)
                    # one store per TWO compute subtiles: doubles the store
                    # descriptor size (the per-subtile ones dip below the
                    # 512B full-rate threshold) at no cost to the final
                    # store's issue time
                    if h % 2 == 1 or h == oh - 1:
                        slo = (h // 2) * 2 * hw
                        st = dict(
                            out=y[0:NV, off:off + npix]
                            .rearrange("c (p x) -> p c x", p=128)[:, :, slo:hi],
                            in_=e_v[:, :, slo:hi],
                        )
                        if last:
                            nc.scalar.dma_start(**st)
                        else:
                            # issued after the NEXT chunk's load emission so
                            # the store's wait-on-DVE never head-of-line-
                            # blocks the load descriptor stream on its engine
                            pending_stores.append(st)

            # software-pipelined emission: chunk s's softmax/output phase is
            # emitted at the end of chunk s, so its DVE chain runs in the DVE
            # idle window while chunk s+1's q/v/K0 load, instead of competing
            # with chunk s+1's multiply-accumulates.  Its stores are deferred
            # to after chunk s+1's load emission (flush inside the K-loop) so
            # their wait-on-DVE never head-of-line-blocks a load engine; exp
            # is the only ACT op in the chain, and it runs while ACT waits
            # for chunk s+1's q to land.
            for s, ncol in enumerate(chunk_cols):
                off, npix = offs[s], 128 * ncol
                x_sl = lambda c0, c1: (
                    x[c0:c1, off:off + npix].rearrange("c (p x) -> p c x", p=128))
                # ---- q
                q_bf = work_pool.tile([128, NK * ncol], bf16, name=f"qbf{s}",
                                      tag="qbf", bufs=2)
                if swdge_k or swdge_qv:
                    nc.gpsimd.dma_start(
                        out=q_bf.rearrange("p (c x) -> p c x", c=NK),
                        in_=x_sl(0, NK))
                else:
                    q_t = in_pool.tile([128, NK * ncol], f32, name=f"q{s}",
                                       tag="q", bufs=qv_bufs)
                    nc.sync.dma_start(
                        out=q_t.rearrange("p (c x) -> p c x", c=NK), in_=x_sl(0, NK))
                    nc.scalar.activation(q_bf, q_t,
                                         mybir.ActivationFunctionType.Copy)
                qbf3 = q_bf.rearrange("p (k x) -> p k x", k=NK)

                def emit_v_load():
                    # v is consumed only at e*v in the NEXT chunk's window, so
                    # it loads after this chunk's K pieces — for the last chunk
                    # that lets the matvec+softmax chain run during v's
                    # transfer instead of after it.
                    if e_bf16 and s + 1 < len(chunk_cols):
                        ev = work_pool.tile([128, NV * ncol], bf16,
                                            name=f"vbf{s}", tag="vbf", bufs=2)
                        if swdge_k or swdge_qv:
                            nc.gpsimd.dma_start(
                                out=ev.rearrange("p (c x) -> p c x", c=NV),
                                in_=x_sl(NK + NK * NV, C))
                        else:
                            v_t = in_pool.tile([128, NV * ncol], f32,
                                               name=f"v{s}", tag="v",
                                               bufs=qv_bufs)
                            nc.sync.dma_start(
                                out=v_t.rearrange("p (c x) -> p c x", c=NV),
                                in_=x_sl(NK + NK * NV, C))
                            nc.scalar.activation(
                                ev, v_t, mybir.ActivationFunctionType.Copy)
                    else:
                        ev = in_pool.tile([128, NV * ncol], f32, name=f"v{s}",
                                          tag="v", bufs=qv_bufs)
                        ring = nc.gpsimd if swdge_k else nc.sync
                        ring.dma_start(
                            out=ev.rearrange("p (c x) -> p c x", c=NV),
                            in_=x_sl(NK + NK * NV, C))
                    return ev

                is_last = s + 1 == len(chunk_cols)
                ev_t = None if is_last else emit_v_load()

                acc = work_pool.tile([128, NV * ncol], bf16, name=f"acc{s}",
                                     tag="acc", bufs=acc_bufs)
                acc3 = acc.rearrange("p (v x) -> p v x", v=NV).unsqueeze(1)
                tmp = work_pool.tile([128, NV * ncol], bf16, name=f"tmp{s}",
                                     tag="tmp", bufs=1)
                tmp3 = tmp.rearrange("p (v x) -> p v x", v=NV).unsqueeze(1)

                for k in range(NK):
                    kbf = work_pool.tile([128, NV * ncol], bf16,
                                         name=f"kbf{s}_{k}", tag="kbf",
                                         bufs=kbf_bufs if swdge_k else 2)
                    if swdge_k:
                        nc.gpsimd.dma_start(
                            out=kbf.rearrange("p (c x) -> p c x", c=NV),
                            in_=x_sl(NK + k * NV, NK + (k + 1) * NV))
                    else:
                        kst = in_pool.tile([128, NV * ncol], f32,
                                           name=f"kst{s}_{k}", tag="kst",
                                           bufs=kst_bufs)
                        nc.sync.dma_start(
                            out=kst.rearrange("p (c x) -> p c x", c=NV),
                            in_=x_sl(NK + k * NV, NK + (k + 1) * NV))
                        nc.scalar.activation(kbf, kst,
                                             mybir.ActivationFunctionType.Copy)
                    kbf3 = kbf.rearrange("p (v x) -> p v x", v=NV).unsqueeze(1)
                    q_bk = qbf3[:, k:k + 1].unsqueeze(2).broadcast_to((128, 1, NV, ncol))
                    last_mac = s + 1 == len(chunk_cols) and k == NK - 1
                    if k == 0:
                        nc.vector.tensor_tensor(acc3, q_bk, kbf3, mybir.AluOpType.mult)
                    elif last_mac:
                        # subtile the final multiply-accumulate so the first
                        # exp quarters can start before the full-width add
                        hw = ncol // out_halves
                        for h in range(out_halves):
                            xs = slice(h * hw, (h + 1) * hw)
                            nc.vector.tensor_tensor(
                                tmp3[:, :, :, xs], q_bk[:, :, :, xs],
                                kbf3[:, :, :, xs], mybir.AluOpType.mult)
                            nc.vector.tensor_tensor(
                                acc3[:, :, :, xs], acc3[:, :, :, xs],
                                tmp3[:, :, :, xs], mybir.AluOpType.add)
                    else:
                        nc.vector.tensor_tensor(tmp3, q_bk, kbf3, mybir.AluOpType.mult)
                        nc.vector.tensor_tensor(acc3, acc3, tmp3, mybir.AluOpType.add)
                    if k == 1:
                        # previous chunk's stores go out behind this chunk's
                        # first loads
                        flush_stores()
                if is_last:
                    ev_t = emit_v_load()
                emit_output_phase(s, acc, ev_t, ncol, last=is_last)
    nc.compile()
    return nc


_NC_CACHE = {}

# v1 build configuration (kept as fallback): bf16 product + add-tree,
# ncol=256 chunks with a tapered first/last chunk.  Measured ~161 us/NEFF.
BUILD_CFG = {
    "recip_on_act": False,
    "bf16_tree": True,
    "k_splits": 8,
    "inplace_tree": True,
    "lean_bufs": True,
    "conv_k": True,
    "direct_b": True,
    "chunk_cols": [192, 256, 256, 256, 64],
}

# default build configuration used by kernel(): v2 super-chunk kernel —
# 2KB DMA descriptors ([512, 384, 128] column taper), K streamed in
# 8-channel pieces (ScalarE f32->bf16 downcast, DVE bf16 multiply-
# accumulate), software-pipelined emission (chunk s-1's softmax/output
# runs inside chunk s's K-loop), quarter-subtiled bf16 output phase with
# SWDGE-cast stores, f32 low-latency output path + HWDGE stores on the
# last chunk.  Measured ~151 us/NEFF median on trn2 (8 cores, ~46.1 MB
# traffic/core at ~358 GB/s HBM/core ≈ 129 us DMA floor); rel-l2 vs the
# f32 reference ≈ 5.5e-3 (bf16 qk tree + bf16 softmax/output path).
BUILD_CFG_V2 = {
    "chunk_cols": [512, 384, 128],
    "e_bf16": True,
    "out_halves": 4,
}


def _get_nc(**cfg):
    if cfg.get("version", 2) == 2:
        cfg = {**BUILD_CFG_V2, **{k: v for k, v in cfg.items() if k != "version"}}
        builder = build_nc_v2
    else:
        cfg = {**BUILD_CFG, **{k: v for k, v in cfg.items() if k != "version"}}
        builder = build_nc
    key = tuple(sorted(
        (k, tuple(v) if isinstance(v, list) else v) for k, v in cfg.items()
    )) + (builder.__name__,)
    if key not in _NC_CACHE:
        _NC_CACHE[key] = builder(**cfg)
    return _NC_CACHE[key]


def make_in_maps(inp):
    in_maps = []
    for core in range(N_CORES):
        b, half = core // 2, core % 2
        shard = np.ascontiguousarray(
            inp[b, :, half * ROWS:(half + 1) * ROWS, :], dtype=np.float32
        ).reshape(C, PIX)
        in_maps.append({"x": shard})
    return in_maps


def assemble_out(results):
    out = np.empty((B, NV, H, W), np.float32)
    for core in range(N_CORES):
        b, half = core // 2, core % 2
        out[b, :, half * ROWS:(half + 1) * ROWS, :] = (
            results[core]["y"].reshape(NV, ROWS, W)
        )
    return out


def run_spmd(inp, trace=False, build_cfg=None, **kwargs):
    """Run the SPMD kernel on 8 cores; returns (full_output, BassKernelResults)."""
    _ensure_path()
    from concourse.bass_utils import run_bass_kernel_spmd

    inp = np.asarray(inp)
    assert inp.shape == (B, C, H, W), inp.shape
    nc = _get_nc(**(build_cfg or {}))
    res = run_bass_kernel_spmd(
        nc, make_in_maps(inp), list(range(N_CORES)), trace=trace, **kwargs
    )
    return assemble_out(res.results), res


def kernel(inp):
    out, _ = run_spmd(inp, trace=False)
    return out

